# revision 32
# baseline (speedup 1.0000x reference)
"""GAT (2-layer, 3-head) forward on 8 Trainium2 NeuronCores.

Sharding: nodes split 8 ways; each core owns 12544 padded destination nodes
and all their incoming edges (1D graph partition per the spec hint). A
channel-major node table (h | a_src | const-1, 16 ch) is replicated into
SBUF as 4 quarters x 2 copies across the 8 GPSIMD 16-partition groups;
per-edge features stream out via ap_gather in dst-canonical slot order.

ap_gather costs ~27ns per index position regardless of payload, so slot
count is minimized: each core's dst nodes are degree-sorted and every
256-dst chunk gets an exact per-chunk window K = ceil(max-quarter-count/2)
(maxed across cores for the shared SPMD graph) — no overflow rows, no fold
pass. Equal-K chunks batch into super-blocks (<=4096 slots) to amortize
per-instruction overhead. Attention logits are exponentiated in a scratch
tile and broadcast across each 16-partition group with ONE block-diagonal
PE matmul; the weighted tile reduces over K-windows on DVE into small
per-block partial tiles, then combines across groups with PE matmuls. The
per-node epilogue trails one block behind the gather stream. Three NEFF
launches: (A) table build (x @ W1aug on PE), (B) edge layer 1 + layer-2
table build, (C) edge layer 2 + head-mean + log_softmax via a PE transpose
so softmax runs on the free axis. Tables are all-gathered between launches
through the host.
"""
import sys
import types

sys.path.insert(0, "/opt/trn_rl_repo")
import numpy as np

N_NODES = 100000
IN_DIM = 256
HID = 3
HEADS = 3
NCLS = 3
NEG = 0.2
EPS = 1e-16

NQ = 4
QREAL = 25000
QN = 25088
NPAD = NQ * QN          # 100352
NCORE = 8
CN = NPAD // NCORE      # 12544
DCHUNK = 256
NCHUNK = CN // DCHUNK   # 49
SCMAX = 4096            # max slots per gather super-block
BMAX = 4                # max 256-dst chunks per super-block
SENT = QREAL
CH = 15
BIG_NEG = -30000.0

# phase-A (table build) chunking
ACHUNK = 448
ANCHUNK = CN // ACHUNK  # 28

LAST_STATS = {}


def _install_ntff_hook_module():
    if "antenv.axon_hooks" in sys.modules:
        return
    mod = types.ModuleType("antenv.axon_hooks")
    state = {"hook": None, "tried": False}

    def set_axon_ntff_profile_hook(hook):
        state["hook"] = hook

    def get_axon_ntff_profile_hook():
        if state["hook"] is None and not state["tried"]:
            state["tried"] = True
            try:
                from trn_agent_boot.trn_boot import _ntff_profile_via_ctypes

                state["hook"] = _ntff_profile_via_ctypes("/opt/axon/libaxon_pjrt.so")
            except Exception:
                state["hook"] = None
        return state["hook"]

    mod.set_axon_ntff_profile_hook = set_axon_ntff_profile_hook
    mod.get_axon_ntff_profile_hook = get_axon_ntff_profile_hook
    sys.modules["antenv.axon_hooks"] = mod


_install_ntff_hook_module()

import concourse.bass as bass
import concourse.mybir as mybir
import concourse.tile as tile
from concourse.bass_utils import run_bass_kernel_spmd
from concourse.library_overlay import lower_extended_insts
from concourse import library_config

F32 = mybir.dt.float32
I16 = mybir.dt.int16
ALU = mybir.AluOpType
ACT = mybir.ActivationFunctionType


def _split_wide_waits(nc):
    """Walrus here caps sync-wait commands per instruction; hoist excess waits
    onto preceding same-engine nofuse NOPs (engines execute in order)."""
    for fn in nc.m.functions:
        for bb in fn.blocks:
            new_insts = []
            for inst in bb.instructions:
                keep = 0 if isinstance(inst, mybir.InstDrain) else 1
                si = inst.sync_info
                if si is not None and si.on_wait is not None and len(si.on_wait) > keep:
                    waits = list(si.on_wait)
                    head, rest = (waits[:-keep], waits[-keep:]) if keep else (waits, [])
                    while head:
                        chunk, head = head[:1], head[1:]
                        nop = mybir.InstNoOp(name=f"I-{nc.next_id()}", ins=[], outs=[])
                        nop.engine = inst.engine
                        nop.bass_nofuse = True
                        nop.sync_info = mybir.SyncInfo(on_wait=chunk, on_update=[])
                        nc.register_instruction(nop, overwrite=True)
                        new_insts.append(nop)
                    inst.sync_info = mybir.SyncInfo(
                        on_wait=rest, on_update=list(si.on_update or [])
                    )
                new_insts.append(inst)
            bb.instructions.clear()
            for i in new_insts:
                bb.add_instruction(i)


def _run(nc, in_maps, trace=False):
    lower_extended_insts(nc)
    _split_wide_waits(nc)
    return run_bass_kernel_spmd(nc, in_maps, core_ids=list(range(NCORE)), trace=trace)


# ---------------------------------------------------------------- launch A
def _build_phase_a():
    nc = bass.Bass("TRN2")
    xT_d = nc.dram_tensor("xT", [IN_DIM, CN], F32, kind="ExternalInput")
    w1_d = nc.dram_tensor("w1", [IN_DIM, HEADS * HID], F32, kind="ExternalInput")
    w1t_d = nc.dram_tensor("w1t", [HEADS * HID, IN_DIM], F32, kind="ExternalInput")
    attw1_d = nc.dram_tensor("attw1", [HEADS * HID, 6], F32, kind="ExternalInput")
    tab_d = nc.dram_tensor("tab", [CH, CN], F32, kind="ExternalOutput")

    with tile.TileContext(nc) as tc:
        with (
            tc.tile_pool(name="const", bufs=1) as cpool,
            tc.tile_pool(name="io", bufs=3) as iopool,
            tc.tile_pool(name="ps", bufs=2, space="PSUM") as pspool,
        ):
            w1aug = cpool.tile([128, 2 * CH], F32)
            w1t = cpool.tile([HEADS * HID, IN_DIM], F32)
            attw1 = cpool.tile([HEADS * HID, 6], F32)
            nc.sync.dma_start(w1t[:], w1t_d[:])
            nc.sync.dma_start(attw1[:], attw1_d[:])
            for k in range(2):
                nc.sync.dma_start(
                    w1aug[:, CH * k:CH * k + 9], w1_d[128 * k:128 * (k + 1), :]
                )
                vps = pspool.tile([128, 6], F32, tag="vps")
                nc.tensor.matmul(
                    out=vps[:],
                    lhsT=w1t[:, 128 * k:128 * (k + 1)],
                    rhs=attw1[:],
                    start=True,
                    stop=True,
                )
                nc.vector.tensor_copy(out=w1aug[:, CH * k + 9:CH * k + 15], in_=vps[:])
            for c in range(ANCHUNK):
                cols = slice(ACHUNK * c, ACHUNK * (c + 1))
                ps = pspool.tile([CH, ACHUNK], F32, tag="ps")
                for k in range(2):
                    xc = iopool.tile([128, ACHUNK], F32, tag="xc")
                    eng = nc.sync if k == 0 else nc.scalar
                    eng.dma_start(xc[:], xT_d[128 * k:128 * (k + 1), cols])
                    nc.tensor.matmul(
                        out=ps[:],
                        lhsT=w1aug[:, CH * k:CH * (k + 1)],
                        rhs=xc[:],
                        start=(k == 0),
                        stop=(k == 1),
                    )
                ob = iopool.tile([CH, ACHUNK], F32, tag="ob")
                nc.vector.tensor_copy(out=ob[:], in_=ps[:])
                nc.sync.dma_start(tab_d[:, cols], ob[:])
    return nc


# ---------------------------------------------------------------- launch B/C
def _build_edge(final, blocks, total_slots):
    nc = bass.Bass("TRN2")
    tab_d = nc.dram_tensor("tabf", [64, QN], F32, kind="ExternalInput")
    idx_d = nc.dram_tensor("idxs", [128, total_slots // 16], I16, kind="ExternalInput")
    adrep_d = nc.dram_tensor("adrep", [128, CN], F32, kind="ExternalInput")
    lhsn_d = nc.dram_tensor("lhsn", [128, 9], F32, kind="ExternalInput")
    lhsd_d = nc.dram_tensor("lhsd", [128, 9], F32, kind="ExternalInput")
    bdiag_d = nc.dram_tensor("bdiag", [128, 128], F32, kind="ExternalInput")
    if final:
        ident9_d = nc.dram_tensor("ident9", [9, 9], F32, kind="ExternalInput")
        b2rep_d = nc.dram_tensor("b2rep", [128, 6], F32, kind="ExternalInput")
        out_d = nc.dram_tensor("outp", [CN, NCLS], F32, kind="ExternalOutput")
    else:
        bias_d = nc.dram_tensor("biasv", [9, 1], F32, kind="ExternalInput")
        w2t_d = nc.dram_tensor("w2t", [9, 9], F32, kind="ExternalInput")
        w2_d = nc.dram_tensor("w2", [9, 9], F32, kind="ExternalInput")
        attw2_d = nc.dram_tensor("attw2", [9, 6], F32, kind="ExternalInput")
        tab2_d = nc.dram_tensor("tab2", [CH, CN], F32, kind="ExternalOutput")

    scmax = max(k * nd for k, nd in blocks)
    ndmax = max(nd for _, nd in blocks)

    with tile.TileContext(nc) as tc:
        with (
            tc.tile_pool(name="big", bufs=1) as bigpool,
            tc.tile_pool(name="gp", bufs=2) as gpool,
            tc.tile_pool(name="wp", bufs=2) as wpool,
            tc.tile_pool(name="io", bufs=3) as iopool,
            tc.tile_pool(name="pr", bufs=3) as prpool,
            tc.tile_pool(name="nd", bufs=2) as ndpool,
            tc.tile_pool(name="psw", bufs=4, space="PSUM") as pswpool,
            tc.tile_pool(name="psn", bufs=1, space="PSUM") as psnpool,
            tc.tile_pool(name="psm", bufs=2, space="PSUM") as psmpool,
        ):
            table = bigpool.tile([128, QN], F32)
            # host ships [64, QN] quarter-major (16 rows per quarter, const-1
            # denominator rows 12..15); one HBM load + SBUF self-copy for the
            # second copy-set (HBM reads run ~90GB/s here, SBUF-SBUF is cheap).
            # two HWDGE rings (sync=qSPDynamicHW, scalar=qActDynamicHW) halve
            # the ~90GB/s single-ring HBM load time
            nc.sync.dma_start(table[0:64, :], tab_d[:])
            nc.scalar.dma_start(table[64:128, :], tab_d[:])
            lhsn = bigpool.tile([128, 9], F32)
            nc.sync.dma_start(lhsn[:], lhsn_d[:])
            lhsd = bigpool.tile([128, 9], F32)
            nc.sync.dma_start(lhsd[:], lhsd_d[:])
            bdiag = bigpool.tile([128, 128], F32)
            nc.sync.dma_start(bdiag[:], bdiag_d[:])
            if final:
                ident9 = bigpool.tile([9, 9], F32)
                nc.sync.dma_start(ident9[:], ident9_d[:])
                b2rep = bigpool.tile([128, 6], F32)
                nc.sync.dma_start(b2rep[:], b2rep_d[:])
            else:
                biasv = bigpool.tile([9, 1], F32)
                nc.sync.dma_start(biasv[:], bias_d[:])
                w2aug = bigpool.tile([9, CH], F32)
                w2t = ndpool.tile([9, 9], F32, tag="sm")
                attw2 = ndpool.tile([9, 6], F32, tag="sm2")
                nc.sync.dma_start(w2t[:], w2t_d[:])
                nc.sync.dma_start(attw2[:], attw2_d[:])
                nc.sync.dma_start(w2aug[:, 0:9], w2_d[:])
                v2ps = psmpool.tile([9, 6], F32, tag="misc")
                nc.tensor.matmul(
                    out=v2ps[:], lhsT=w2t[:], rhs=attw2[:], start=True, stop=True
                )
                nc.vector.tensor_copy(out=w2aug[:, 9:15], in_=v2ps[:])

            tab_in = table[:].rearrange("p (n d) -> p n d", d=1)
            nc.gpsimd.load_library(library_config.ap_gather)

            def node_body(par, d0, n0):
                """per-256-dst epilogue; par holds this block's partials,
                columns [n0, n0+256) of it; d0+n0 = global dst offset."""
                rhs = par[:, n0:n0 + DCHUNK]
                ncol = slice(d0 + n0, d0 + n0 + DCHUNK)
                ndn_ps = psnpool.tile([9, DCHUNK], F32, tag="ndn")
                ndd_ps = psnpool.tile([9, DCHUNK], F32, tag="ndd")
                nc.tensor.matmul(
                    out=ndn_ps[:], lhsT=lhsn[:], rhs=rhs, start=True, stop=True
                )
                nc.tensor.matmul(
                    out=ndd_ps[:], lhsT=lhsd[:], rhs=rhs, start=True, stop=True
                )
                rden9 = ndpool.tile([9, DCHUNK], F32, tag="rden")
                nc.vector.tensor_scalar_add(out=rden9[:], in0=ndd_ps[:], scalar1=EPS)
                nc.vector.reciprocal(out=rden9[:], in_=rden9[:])
                hagg = ndpool.tile([9, DCHUNK], F32, tag="hagg")
                nc.vector.tensor_tensor(
                    out=hagg[:], in0=ndn_ps[:], in1=rden9[:], op=ALU.mult
                )
                if not final:
                    nc.vector.tensor_tensor(
                        out=hagg[:], in0=hagg[:],
                        in1=biasv[:].to_broadcast([9, DCHUNK]), op=ALU.add,
                    )
                    t1 = ndpool.tile([9, DCHUNK], F32, tag="t1")
                    nc.vector.tensor_scalar_min(out=t1[:], in0=hagg[:], scalar1=0.0)
                    nc.scalar.activation(out=t1[:], in_=t1[:], func=ACT.Exp)
                    # elu = relu(x) + exp(min(x,0)) - 1
                    nc.vector.tensor_scalar_max(out=hagg[:], in0=hagg[:], scalar1=0.0)
                    nc.vector.tensor_tensor(
                        out=hagg[:], in0=hagg[:], in1=t1[:], op=ALU.add
                    )
                    nc.vector.tensor_scalar_add(out=hagg[:], in0=hagg[:], scalar1=-1.0)
                    t2ps = psmpool.tile([CH, DCHUNK], F32, tag="misc")
                    nc.tensor.matmul(
                        out=t2ps[:], lhsT=w2aug[:], rhs=hagg[:], start=True, stop=True
                    )
                    t2sb = ndpool.tile([CH, DCHUNK], F32, tag="t2sb")
                    nc.vector.tensor_copy(out=t2sb[:], in_=t2ps[:])
                    nc.sync.dma_start(tab2_d[:, ncol], t2sb[:])
                else:
                    tp = psmpool.tile([128, 18], F32, tag="misc")
                    for m in range(2):
                        nc.tensor.transpose(
                            tp[:, 9 * m:9 * (m + 1)],
                            hagg[:, 128 * m:128 * (m + 1)],
                            ident9[:],
                        )
                    zm = ndpool.tile([128, 6], F32, tag="zm")
                    nc.vector.tensor_reduce(
                        out=zm[:].rearrange("p (n c) -> p n c", c=3),
                        in_=tp[:].rearrange("p (n h c) -> p n c h", h=3, c=3),
                        axis=mybir.AxisListType.X,
                        op=ALU.add,
                    )
                    nc.vector.tensor_scalar_mul(
                        out=zm[:], in0=zm[:], scalar1=1.0 / HEADS
                    )
                    nc.vector.tensor_tensor(
                        out=zm[:], in0=zm[:], in1=b2rep[:], op=ALU.add
                    )
                    ez = ndpool.tile([128, 6], F32, tag="ez")
                    nc.scalar.activation(out=ez[:], in_=zm[:], func=ACT.Exp)
                    sz = ndpool.tile([128, 2], F32, tag="sz")
                    nc.vector.tensor_reduce(
                        out=sz[:],
                        in_=ez[:].rearrange("p (n c) -> p n c", c=3),
                        axis=mybir.AxisListType.X,
                        op=ALU.add,
                    )
                    nc.scalar.activation(out=sz[:], in_=sz[:], func=ACT.Ln)
                    zf = ndpool.tile([128, 6], F32, tag="zf")
                    nc.vector.tensor_tensor(
                        out=zf[:].rearrange("p (n c) -> p n c", c=3),
                        in0=zm[:].rearrange("p (n c) -> p n c", c=3),
                        in1=sz[:].to_broadcast([128, 2, 3]),
                        op=ALU.subtract,
                    )
                    nc.sync.dma_start(
                        out_d[ncol, :].rearrange("(m p) c -> p m c", p=128),
                        zf[:].rearrange("p (m c) -> p m c", c=3),
                    )

            prev = None  # (par tile, dst offset, ndst) of previous block
            soff = 0
            doff = 0
            for bi, (K, nd) in enumerate(blocks):
                SC = K * nd
                if prev is not None:
                    p_par, p_doff, p_nd = prev
                    for n0 in range(0, p_nd, DCHUNK):
                        node_body(p_par, p_doff, n0)
                # prefetches ride the vector engine's DMA path so they don't
                # fragment the sync/scalar HWDGE rings carrying the table load
                idxc = iopool.tile(
                    [128, SC // 16], I16, tag="idxc", padded_shape=[128, scmax // 16]
                )
                nc.gpsimd.dma_start(idxc[:], idx_d[:, soff // 16:(soff + SC) // 16])
                adc = iopool.tile(
                    [128, nd], F32, tag="adc", padded_shape=[128, ndmax]
                )
                nc.gpsimd.dma_start(adc[:], adrep_d[:, doff:doff + nd])
                g_t = gpool.tile([128, SC], F32, tag="g", padded_shape=[128, scmax])
                nc.gpsimd.ap_gather(
                    out_ap=g_t[:].rearrange("p (n d) -> p n d", d=1),
                    in_ap=tab_in,
                    idxs_ap=idxc[:],
                    channels=128,
                    num_elems=QN,
                    d=1,
                    num_idxs=SC,
                )
                wt = wpool.tile([128, SC], F32, tag="wt", padded_shape=[128, scmax])
                nc.vector.tensor_tensor(
                    out=wt[:].rearrange("p (n j) -> p n j", j=K),
                    in0=g_t[:].rearrange("p (n j) -> p n j", j=K),
                    in1=adc[:].to_broadcast([128, nd, K]),
                    op=ALU.add,
                )
                # leaky relu then exp
                nc.vector.scalar_tensor_tensor(
                    out=wt[:], in0=wt[:], scalar=NEG, in1=wt[:],
                    op0=ALU.mult, op1=ALU.max,
                )
                nc.scalar.activation(out=wt[:], in_=wt[:], func=ACT.Exp)
                for lo in range(0, SC, 512):
                    w = min(512, SC - lo)
                    wps = pswpool.tile([128, 512], F32, tag="w")
                    nc.tensor.matmul(
                        out=wps[:, 0:w],
                        lhsT=bdiag[:],
                        rhs=wt[:, lo:lo + w],
                        start=True,
                        stop=True,
                    )
                    nc.vector.tensor_tensor(
                        out=g_t[:, lo:lo + w],
                        in0=g_t[:, lo:lo + w],
                        in1=wps[:, 0:w],
                        op=ALU.mult,
                    )
                par = prpool.tile([128, nd], F32, tag="par", padded_shape=[128, ndmax])
                nc.vector.tensor_reduce(
                    out=par[:],
                    in_=g_t[:].rearrange("p (n j) -> p n j", j=K),
                    axis=mybir.AxisListType.X,
                    op=ALU.add,
                )
                prev = (par, doff, nd)
                soff += SC
                doff += nd
            p_par, p_doff, p_nd = prev
            for n0 in range(0, p_nd, DCHUNK):
                node_body(p_par, p_doff, n0)
    return nc


# ---------------------------------------------------------------- host side
def _balance_quarters(src, dst):
    """Greedy batched assignment of nodes to the 4 quarters minimizing
    sum_d ceil(max-quarter-in-count(d)/2) — the slot-grid size. Returns
    pos[n] = padded slot id of original node n."""
    rng = np.random.default_rng(7)
    order_e = np.argsort(src, kind="stable")
    s_sorted = src[order_e]
    d_sorted = dst[order_e].astype(np.int64)
    deg = np.bincount(src, minlength=N_NODES)
    ptr = np.r_[0, np.cumsum(deg)]
    proc = np.argsort(-deg, kind="stable")

    cnt = np.zeros((N_NODES, 4), np.int32)
    quota = np.full(4, QREAL, np.int64)
    qassign = np.full(N_NODES, -1, np.int8)
    B = 256
    for p in range(5):
        for b0 in range(0, N_NODES, B):
            nodes = proc[b0:b0 + B]
            degs = deg[nodes]
            starts = ptr[nodes]
            idx = np.repeat(
                starts - np.r_[0, np.cumsum(degs)[:-1]], degs
            ) + np.arange(int(degs.sum()))
            D = d_sorted[idx]
            if p > 0:  # remove current contribution, then re-decide
                qe_old = qassign[np.repeat(nodes, degs)].astype(np.int64)
                np.subtract.at(cnt, (D, qe_old), 1)
                for q in range(4):
                    quota[q] += int((qassign[nodes] == q).sum())
                qassign[nodes] = -1
            c = cnt[D]                                # [L, 4]
            m = c.max(1)
            base = (m + 1) // 2
            newmax = np.maximum(m[:, None], c + 1)
            delta = ((newmax + 1) // 2 - base[:, None]).astype(np.float32)
            delta += 0.02 * (c - c.mean(1, keepdims=True))
            bounds = np.r_[0, np.cumsum(degs)][:-1]
            cost = np.add.reduceat(delta, bounds, axis=0)
            cost += rng.random(cost.shape) * 1e-3     # tie-break
            # capacity-aware assignment
            remaining = np.ones(len(nodes), bool)
            for _ in range(4):
                open_q = np.flatnonzero(quota > 0)
                if not remaining.any():
                    break
                sub = np.flatnonzero(remaining)
                pick = open_q[np.argmin(cost[sub][:, open_q], axis=1)]
                for q in open_q:
                    mine = sub[pick == q]
                    take = mine[:int(quota[q])]
                    qassign[nodes[take]] = q
                    quota[q] -= len(take)
                    remaining[take] = False
                    if len(take) < len(mine):
                        cost[mine[len(take):], q] = np.inf
            qe = qassign[np.repeat(nodes, degs)]
            np.add.at(cnt, (D, qe.astype(np.int64)), 1)
    assert (qassign >= 0).all() and (quota == 0).all()

    pos = np.empty(N_NODES, np.int64)
    for q in range(4):
        members = np.flatnonzero(qassign == q)
        members = members[rng.permutation(len(members))]
        pos[members] = q * QN + np.arange(len(members))
    return pos


def _wrap_chunked(stream, chunk):
    """[G, S] streams -> [16G, S//16] ap_gather idx layout, wrapped per chunk."""
    g, s = stream.shape
    nch = s // chunk
    w = stream.reshape(g, nch, chunk // 16, 16)
    w = w.transpose(0, 3, 1, 2)
    return np.ascontiguousarray(w.reshape(g * 16, s // 16))


def _pack_edges(srcN, dstN):
    core = dstN // CN
    dloc = dstN % CN
    q = srcN // QN
    sloc = (srcN % QN).astype(np.int16)

    # per-(core,dst,quarter) counts -> exact per-dst window size K
    cnt = np.bincount(
        (core * CN + dloc) * 4 + q, minlength=NCORE * CN * 4
    ).reshape(NCORE, CN, 4)
    maxq = cnt.max(2)
    Kd = (maxq + 1) // 2                       # [NCORE, CN], >=1 (self loop)
    perms = np.argsort(-Kd, axis=1, kind="stable")
    Ks = np.take_along_axis(Kd, perms, 1)
    Kch = Ks[:, ::DCHUNK].max(0).astype(int)   # [NCHUNK] shared SPMD profile

    # batch equal-K chunk runs into super-blocks
    blocks = []
    i = 0
    while i < NCHUNK:
        K = int(Kch[i])
        j = i + 1
        while (
            j < NCHUNK and int(Kch[j]) == K and (j - i + 1) * DCHUNK * K <= SCMAX
            and (j - i) < BMAX
        ):
            j += 1
        blocks.append((K, (j - i) * DCHUNK))
        i = j
    # per-chunk lookup arrays
    chunk_block = np.empty(NCHUNK, int)
    b0 = 0
    dst0 = np.empty(len(blocks), int)
    kblk = np.empty(len(blocks), int)
    soff = np.empty(len(blocks), int)
    s = 0
    d0 = 0
    for bi, (K, nd) in enumerate(blocks):
        nch = nd // DCHUNK
        chunk_block[b0:b0 + nch] = bi
        dst0[bi] = d0
        kblk[bi] = K
        soff[bi] = s
        s += K * nd
        d0 += nd
        b0 += nch
    total_slots = s

    # rank of each dst within its core's degree-sorted order
    ranks = np.empty_like(perms)
    np.put_along_axis(ranks, perms, np.broadcast_to(np.arange(CN), (NCORE, CN)), 1)

    # rank of each edge within its (core,dst,quarter) group
    key = (core * CN + dloc) * 4 + q
    order = np.argsort(key, kind="stable")
    ks = key[order]
    grp_start = np.r_[0, np.flatnonzero(np.diff(ks)) + 1]
    sizes = np.diff(np.r_[grp_start, len(ks)])
    rank = np.arange(len(ks)) - np.repeat(grp_start, sizes)

    co, dl, qo, sl = core[order], dloc[order], q[order], sloc[order]
    rk = ranks[co, dl]
    bid = chunk_block[rk // DCHUNK]
    base = soff[bid] + (rk - dst0[bid]) * kblk[bid]
    g = qo + 4 * (rank & 1)
    k = rank >> 1
    assert (k < kblk[bid]).all(), "slot capacity exceeded"

    streams = np.full((NCORE, 8, total_slots), SENT, dtype=np.int16)
    streams[co, g, base + k] = sl

    idx_wr = np.stack(
        [
            np.concatenate(
                [
                    _wrap_chunked(
                        streams[c][:, soff[bi]:soff[bi] + K * nd], K * nd
                    )
                    for bi, (K, nd) in enumerate(blocks)
                ],
                axis=1,
            )
            for c in range(NCORE)
        ]
    )
    return idx_wr, perms, blocks, total_slots


def kernel(x, edge_index, W1, att_src1, att_dst1, b1, W2, att_src2, att_dst2, b2):
    import os as _os
    import time as _time

    x = np.asarray(x, np.float32)
    W1 = np.asarray(W1, np.float32)
    W2 = np.asarray(W2, np.float32)
    b1v = np.asarray(b1, np.float32)
    b2v = np.asarray(b2, np.float32)

    loops = np.arange(N_NODES, dtype=np.int64)
    src = np.concatenate([np.asarray(edge_index[0], np.int64), loops])
    dst = np.concatenate([np.asarray(edge_index[1], np.int64), loops])
    pos = _balance_quarters(src, dst)
    idx_wr, perms, blocks, total_slots = _pack_edges(pos[src], pos[dst])

    xP = np.zeros((NPAD, IN_DIM), np.float32)
    xP[pos] = x
    xT = np.ascontiguousarray(xP.T)

    def tab16(tab):
        t = np.ones((16, NPAD), np.float32)
        t[0:12] = tab[0:12]
        # -> [64, QN] quarter-major: row 16q+j = channel j of quarter q
        return np.ascontiguousarray(
            t.reshape(16, 4, QN).transpose(1, 0, 2).reshape(64, QN)
        )

    def attw(att_s, att_d):
        a = np.zeros((HEADS * HID, 6), np.float32)
        for h in range(HEADS):
            for cc in range(3):
                a[3 * h + cc, h] = np.asarray(att_s, np.float32)[h, cc]
                a[3 * h + cc, 3 + h] = np.asarray(att_d, np.float32)[h, cc]
        return a

    attw1 = attw(att_src1, att_dst1)
    attw2 = attw(att_src2, att_dst2)

    lhsn = np.zeros((128, 9), np.float32)
    lhsd = np.zeros((128, 9), np.float32)
    bdiag = np.zeros((128, 128), np.float32)
    for p in range(128):
        j = p % 16
        g16 = p - j
        if j < 9:
            lhsn[p, j] = 1.0
        elif j < 12:
            h = j - 9
            for cc in range(3):
                bdiag[p, g16 + 3 * h + cc] = 1.0
            bdiag[p, g16 + 12 + h] = 1.0
        elif j < 15:
            for cc in range(3):
                lhsd[p, 3 * (j - 12) + cc] = 1.0
    ident9 = np.eye(9, dtype=np.float32)
    b2rep = np.tile(b2v.reshape(1, NCLS), (128, 2)).astype(np.float32)
    b1m = b1v.reshape(9, 1).copy()

    def make_adrep(tab):
        out = []
        for c in range(NCORE):
            ad = tab[12:15, CN * c:CN * (c + 1)][:, perms[c]]
            rep = np.zeros((128, CN), np.float32)
            for g in range(8):
                rep[16 * g + 9:16 * g + 12, :] = ad
            out.append(rep)
        return out

    trace = bool(int(_os.environ.get("KERNEL_TRACE", "0")))
    stats = {}
    t0 = _time.time()

    ncA = _build_phase_a()
    in_maps = [
        {
            "xT": np.ascontiguousarray(xT[:, CN * c:CN * (c + 1)]),
            "w1": W1,
            "w1t": np.ascontiguousarray(W1.T),
            "attw1": attw1,
        }
        for c in range(NCORE)
    ]
    resA = _run(ncA, in_maps, trace=trace)
    stats["A_ns"] = resA.exec_time_ns
    tab1 = np.concatenate([resA.results[c]["tab"] for c in range(NCORE)], axis=1)
    padmask = np.zeros(NPAD, bool)
    for qq in range(NQ):
        padmask[QN * qq + QREAL:QN * (qq + 1)] = True
    tab1[9:12, padmask] = BIG_NEG

    ncB = _build_edge(False, blocks, total_slots)
    adreps = make_adrep(tab1)
    tab1w = tab16(tab1)
    in_maps = [
        {
            "tabf": tab1w,
            "idxs": idx_wr[c],
            "adrep": adreps[c],
            "lhsn": lhsn,
            "lhsd": lhsd,
            "bdiag": bdiag,
            "biasv": b1m,
            "w2t": np.ascontiguousarray(W2.T),
            "w2": W2,
            "attw2": attw2,
        }
        for c in range(NCORE)
    ]
    resB = _run(ncB, in_maps, trace=trace)
    stats["B_ns"] = resB.exec_time_ns
    # tab2 columns come out in perm order; un-permute to local dst order
    tab2 = np.empty((CH, NPAD), np.float32)
    for c in range(NCORE):
        tab2[:, CN * c + perms[c]] = resB.results[c]["tab2"]
    tab2[9:12, padmask] = BIG_NEG

    ncC = _build_edge(True, blocks, total_slots)
    adreps = make_adrep(tab2)
    tab2w = tab16(tab2)
    in_maps = [
        {
            "tabf": tab2w,
            "idxs": idx_wr[c],
            "adrep": adreps[c],
            "lhsn": lhsn,
            "lhsd": lhsd,
            "bdiag": bdiag,
            "ident9": ident9,
            "b2rep": b2rep,
        }
        for c in range(NCORE)
    ]
    resC = _run(ncC, in_maps, trace=trace)
    stats["C_ns"] = resC.exec_time_ns
    outP = np.empty((NPAD, NCLS), np.float32)
    for c in range(NCORE):
        outP[CN * c + perms[c]] = resC.results[c]["outp"]
    stats["wall_s"] = _time.time() - t0

    out = outP[pos]
    LAST_STATS.clear()
    LAST_STATS.update(stats)
    return np.ascontiguousarray(out, dtype=np.float32)
